# revision 34
# baseline (speedup 1.0000x reference)
"""Trainium2 Bass kernel for banded continuous-conv1d (sparse_attention).

Math (per batch b, position i, K=16 band offsets d=1..K):
    dt[b,i,d] = relu(t_i - t_{i-d})           (masked where i-d < 0)
    h1 = relu(dt @ W1 + b1)                   (scalar -> 128)
    h2 = relu(h1 @ W2 + b2)                   (128 -> 128)
    kv = (h2 @ W3 + b3) masked                (128 -> 32*32)
    out[b,i,o] = sum_{d,c} feat[b,i,c] * kv[b,i,d,c,o]

Key factorization (exact, any biases): features don't depend on d, so the
d-sum moves before the W3 matmul:
    H[h,(b,i)]   = sum_d h2[b,i,d,h]
    KV[(b,i),co] = sum_h H[h,(b,i)] * W3[h,co]  (+ rank-1 bias/mask corrections)
    out[(b,i),o] = sum_c feat[(b,i),c] * KV[(b,i), c*32+o]
This cuts the W3-stage FLOPs by 16x. Bias/mask corrections are rank-1 in q
and folded into the KV PSUM accumulation as a K=2 matmul
(rows: nv*b3 and -(K-nv)*kv0, kv0 = MLP(0) @ W3), so results stay exact
for arbitrary b1/b2/b3 even though dt-masking zeroes invalid offsets.

Sharding: 8 cores = 2 batches x 4 sequence shards of 512 positions; each
shard's tB carries a 16-timestamp halo from the previous shard (host-staged).
"""

import sys

import numpy as np

sys.path.insert(0, "/opt/trn_rl_repo")

from concourse import bacc, bass, mybir, tile  # noqa: E402
from concourse.bass_utils import run_bass_kernel_spmd  # noqa: E402

BS, L, CIN, COUT, HID, K = 2, 2048, 32, 32, 128, 16
NCORES = 8
NSH = 4          # sequence shards per batch
SH = L // NSH    # positions per core (512)
NQT = SH // 128  # q-tiles per core (4)
F32 = mybir.dt.float32

_cache: dict = {}


def _enable_ldw_opt():
    """Let walrus dedup identical consecutive LDWEIGHTS (the default
    --enable-ldw-opt=false re-loads the stationary operand before every
    matmul; our phases reuse one stationary operand dozens of times)."""
    from concourse import bass_utils

    if getattr(bass_utils.run_command, "_ldw_patched", False):
        return
    orig = bass_utils.run_command

    def patched(cmd, *a, **kw):
        cmd = [
            c.replace("--enable-ldw-opt=false", "--enable-ldw-opt=true")
            if isinstance(c, str) else c
            for c in cmd
        ]
        return orig(cmd, *a, **kw)

    patched._ldw_patched = True
    bass_utils.run_command = patched


def _build_bass():
    """Build + compile the SPMD single-core Bass program (identical on all
    cores; per-core behavior comes entirely from the input tensors)."""
    _enable_ldw_opt()
    nc = bacc.Bacc("TRN2", target_bir_lowering=False, debug=False)

    R32 = mybir.dt.float32r  # fp32 bits, single-pass PE mode (1 cyc/row vs 4)
    dram = {}
    for name, shape, dt_ in [
        ("tA", (K, SH), F32),       # t_i broadcast over d rows
        ("tB", (K, SH), F32),       # t_{i-1-d}, halo-padded (clipped to t_0)
        ("mask16", (K, SH), F32),   # 1.0 where i-1-d >= 0
        ("featq", (128, NQT * CIN), F32),  # feat[q, t*32+c] (q-tile-major)
        ("W1r", (1, HID), R32),     # W1 row
        ("W2", (HID, HID), R32),
        ("W3", (HID, CIN * COUT), R32),
        ("b1c", (HID, 1), F32),
        ("b2c", (HID, 1), F32),
        ("eye", (HID, HID), R32),   # identity for the d-sum PSUM accumulation
        ("nvmat", (2, SH), R32),    # rows: nv, K-nv  (valid-offset counts)
        ("rhs2", (2, CIN * COUT), R32),  # rows: b3, -kv0
    ]:
        dram[name] = nc.dram_tensor(name, list(shape), dt_, kind="ExternalInput")
    out_dram = nc.dram_tensor("out", [SH, COUT], F32, kind="ExternalOutput")

    Relu = mybir.ActivationFunctionType.Relu
    Add = mybir.AluOpType.add
    Max = mybir.AluOpType.max
    Mult = mybir.AluOpType.mult



    NW = 1024  # wide tile: 2 d-offsets side by side (2 PSUM banks)

    with tile.TileContext(nc) as tc:
        with (
            tc.tile_pool(name="const", bufs=1) as const,
            tc.tile_pool(name="work", bufs=1) as work,
            tc.tile_pool(name="h1p", bufs=8) as h1p,
            tc.tile_pool(name="h2p", bufs=8) as h2p,
            tc.tile_pool(name="stage5", bufs=2) as s5p,
            # 2 pools x 2 bufs x [128,1024] = 8 PSUM banks total; the H
            # accumulator and KV tiles reuse these slots after the phases.
            tc.tile_pool(name="ps1", bufs=2, space=bass.MemorySpace.PSUM) as ps1,
            tc.tile_pool(name="ps2", bufs=2, space=bass.MemorySpace.PSUM) as ps2,
        ):
            # ---- load inputs, spread over engine DMA queues ----
            # dt-critical inputs go first on their queues
            qeng = {
                "tA": nc.sync, "tB": nc.sync, "mask16": nc.sync,
                "W1r": nc.scalar, "b1c": nc.scalar, "W2": nc.scalar,
                "b2c": nc.scalar, "eye": nc.scalar,
                "W3": nc.gpsimd, "featq": nc.gpsimd,
                "nvmat": nc.gpsimd, "rhs2": nc.gpsimd,
            }
            sb = {}
            for name in qeng:
                t = const.tile(list(dram[name].shape), dram[name].dtype, tag=name)
                qeng[name].dma_start(t[:], dram[name].ap())
                sb[name] = t

            # ---- dt = relu(tA - tB) * mask ----
            dtsub = work.tile([K, SH], F32, tag="dtsub")
            nc.vector.tensor_sub(dtsub[:], sb["tA"][:], sb["tB"][:])
            dt2 = work.tile([K, SH], R32, tag="dt2")
            nc.vector.scalar_tensor_tensor(
                dt2[:], dtsub[:], 0.0, sb["mask16"][:], op0=Max, op1=Mult
            )
            # gather all 16 d-rows into one partition-0 tile (matmul operands
            # must start at a 32-aligned partition): drow d = dtrow[:, d*SH:]
            dtrow = work.tile([1, K * SH], R32, tag="dtrow")
            nc.sync.dma_start(
                dtrow[:].rearrange("p (d q) -> p d q", d=K), dt2[:, :]
            )
            drows = [dtrow[:, d * SH : (d + 1) * SH] for d in range(K)]

            # ---- PE warm-up: dead-time zero matmuls so the HAM clock gate
            # opens (4/8 -> 8/8) before the real stream begins ----
            wzf = work.tile([HID, SH], F32, tag="wzf")
            nc.gpsimd.memset(wzf[:], 0.0)
            wz = work.tile([HID, SH], R32, tag="wz")
            nc.vector.tensor_copy(wz[:], wzf[:])
            for i in range(14):
                pw = ps1.tile([HID, NW], F32, tag="p1")
                nc.tensor.matmul(
                    pw[:, :SH], wz[:, :HID], wz[:], start=True, stop=True
                )

            # ---- per-offset MLP, phase-separated (constant stationary
            # operand per phase keeps the PE stream dense), processed in
            # d-pairs so relus run as wide [128,1024] ops ----
            # Phase A: h1_d = relu(W1 (x) dt_d + b1)
            h1s = []
            for p in range(K // 2):
                pA = ps1.tile([HID, NW], F32, tag="p1")
                for j in range(2):
                    nc.tensor.matmul(
                        pA[:, j * SH : (j + 1) * SH], sb["W1r"][:],
                        drows[2 * p + j], start=True, stop=True,
                    )
                h1 = h1p.tile([HID, NW], R32, tag="h1")
                if p in (0, 2, 4, 5, 6):
                    nc.scalar.activation(h1[:], pA[:], Relu, bias=sb["b1c"][:])
                else:
                    nc.vector.tensor_scalar(
                        h1[:], pA[:], sb["b1c"][:], 0.0, op0=Add, op1=Max
                    )
                h1s.append(h1)
            # Phase B: h2_d = relu(W2.T @ h1_d + b2)
            h2s = []
            for p in range(K // 2):
                pB = ps2.tile([HID, NW], F32, tag="p2")
                for j in range(2):
                    nc.tensor.matmul(
                        pB[:, j * SH : (j + 1) * SH], sb["W2"][:],
                        h1s[p][:, j * SH : (j + 1) * SH], start=True, stop=True,
                    )
                h2 = h2p.tile([HID, NW], R32, tag="h2")
                if p in (1, 3, 5, 6, 7):
                    nc.scalar.activation(h2[:], pB[:], Relu, bias=sb["b2c"][:])
                else:
                    nc.vector.tensor_scalar(
                        h2[:], pB[:], sb["b2c"][:], 0.0, op0=Add, op1=Max
                    )
                h2s.append(h2)
            # Phase C: H = sum_d h2_d (identity matmuls accumulating in PSUM).
            # The accumulator reuses a ps1 slot (phase A is drained by now).
            pHw = ps1.tile([HID, NW], F32, tag="p1")
            pH = pHw[:, :SH]
            n = 0
            for p in range(K // 2):
                for j in range(2):
                    nc.tensor.matmul(
                        pH, sb["eye"][:], h2s[p][:, j * SH : (j + 1) * SH],
                        start=(n == 0), stop=(n == K - 1),
                    )
                    n += 1

            Hs = work.tile([HID, SH], R32, tag="Hs")
            nc.vector.tensor_copy(Hs[:], pH)

            # ---- KV = H^T @ W3 (+ rank-1 corrections), then f-contraction ----
            CO = CIN * COUT
            for t in range(NQT):
                qs = slice(t * 128, (t + 1) * 128)
                kv = (ps2 if t % 2 == 0 else ps1).tile(
                    [128, CO], F32, tag="p2" if t % 2 == 0 else "p1"
                )
                for half in range(2):
                    hs = slice(half * 512, half * 512 + 512)
                    nc.tensor.matmul(
                        kv[:, hs], Hs[:, qs], sb["W3"][:, hs],
                        start=True, stop=False,
                    )
                for half in range(2):
                    hs = slice(half * 512, half * 512 + 512)
                    nc.tensor.matmul(
                        kv[:, hs], sb["nvmat"][:, qs], sb["rhs2"][:, hs],
                        start=False, stop=True,
                    )
                # prod stored o-major: prod[q, o*32+c] = kv[q, c*32+o]*f[q,c]
                # so the c-reduction below reads contiguously. Tiles 2,3 run
                # the multiply on gpsimd (needs an SBUF copy of kv first,
                # done by the otherwise-idle ACT engine).
                prod = s5p.tile([128, CO], F32, tag="prod")
                fview = (
                    sb["featq"][:, t * CIN : (t + 1) * CIN]
                    .unsqueeze(2)
                    .broadcast_to([128, CIN, COUT])
                )
                if t < 2:
                    nc.vector.tensor_tensor(
                        prod[:].rearrange("p (o c) -> p c o", c=CIN),
                        kv[:].rearrange("p (c o) -> p c o", o=COUT),
                        fview,
                        op=Mult,
                    )
                else:
                    kvs = s5p.tile([128, CO], F32, tag="kvs")
                    nc.scalar.copy(kvs[:], kv[:])
                    nc.gpsimd.tensor_tensor(
                        prod[:].rearrange("p (o c) -> p c o", c=CIN),
                        kvs[:].rearrange("p (c o) -> p c o", o=COUT),
                        fview,
                        op=Mult,
                    )
                # out[q, o] = sum_c prod[q, o, c]
                ot = s5p.tile([128, COUT], F32, tag="ot")
                nc.vector.tensor_reduce(
                    ot[:],
                    prod[:].rearrange("p (o c) -> p o c", c=CIN),
                    axis=mybir.AxisListType.X,
                    op=Add,
                )
                nc.sync.dma_start(out_dram.ap()[qs, :], ot[:])

    nc.compile()
    return nc


def _stage_inputs(times, features, W1, b1, W2, b2, W3, b3):
    """Host-side staging: shard + precompute per-core input tensors."""
    times = np.ascontiguousarray(times, dtype=np.float32)
    features = np.ascontiguousarray(features, dtype=np.float32)
    W1 = np.asarray(W1, np.float32).reshape(1, HID)
    b1 = np.asarray(b1, np.float32).reshape(HID)
    W2 = np.asarray(W2, np.float32)
    b2 = np.asarray(b2, np.float32).reshape(HID)
    W3 = np.asarray(W3, np.float32)
    b3 = np.asarray(b3, np.float32).reshape(CIN * COUT)

    h2_0 = np.maximum(W2.T @ np.maximum(b1, 0.0) + b2, 0.0)
    kv0 = h2_0 @ W3
    rhs2 = np.ascontiguousarray(np.stack([b3, -kv0]).astype(np.float32))
    eye = np.eye(HID, dtype=np.float32)
    b1c = np.ascontiguousarray(b1[:, None])
    b2c = np.ascontiguousarray(b2[:, None])
    dd = np.arange(K)[:, None]

    in_maps = []
    for c in range(NCORES):
        b, s = divmod(c, NSH)
        gi = s * SH + np.arange(SH)
        src = gi[None, :] - 1 - dd
        in_maps.append({
            "tA": np.ascontiguousarray(np.broadcast_to(times[b, gi], (K, SH))),
            "tB": np.ascontiguousarray(times[b, np.clip(src, 0, L - 1)]),
            "mask16": (src >= 0).astype(np.float32),
            "featq": np.ascontiguousarray(
                features[b, gi].reshape(NQT, 128, CIN)
                .transpose(1, 0, 2).reshape(128, NQT * CIN)
            ),
            "W1r": W1,
            "W2": W2,
            "W3": W3,
            "b1c": b1c,
            "b2c": b2c,
            "eye": eye,
            "nvmat": np.ascontiguousarray(
                np.stack([np.minimum(gi, K), K - np.minimum(gi, K)])
            ).astype(np.float32),
            "rhs2": rhs2,
        })
    return in_maps


def kernel(times, features, W1, b1, W2, b2, W3, b3, kernel_size, **run_kwargs):
    assert int(kernel_size) == K
    assert times.shape == (BS, L) and features.shape == (BS, L, CIN)

    if "nc" not in _cache:
        _cache["nc"] = _build_bass()
    nc = _cache["nc"]

    in_maps = _stage_inputs(times, features, W1, b1, W2, b2, W3, b3)
    res = run_bass_kernel_spmd(
        nc, in_maps, core_ids=list(range(NCORES)), **run_kwargs
    )

    out = np.empty((BS, L, COUT), np.float32)
    for c in range(NCORES):
        b, s = divmod(c, NSH)
        out[b, s * SH : (s + 1) * SH, :] = res.results[c]["out"]
    if run_kwargs:
        _cache["last_results"] = res
    return out


# revision 41
# speedup vs baseline: 1.0405x; 1.0405x over previous
"""Trainium2 Bass kernel for banded continuous-conv1d (sparse_attention).

Math (per batch b, position i, K=16 band offsets d=1..K):
    dt[b,i,d] = relu(t_i - t_{i-d})           (masked where i-d < 0)
    h1 = relu(dt @ W1 + b1)                   (scalar -> 128)
    h2 = relu(h1 @ W2 + b2)                   (128 -> 128)
    kv = (h2 @ W3 + b3) masked                (128 -> 32*32)
    out[b,i,o] = sum_{d,c} feat[b,i,c] * kv[b,i,d,c,o]

Key factorization (exact, any biases): features don't depend on d, so the
d-sum moves before the W3 matmul:
    H[h,(b,i)]   = sum_d h2[b,i,d,h]
    KV[(b,i),co] = sum_h H[h,(b,i)] * W3[h,co]  (+ rank-1 bias/mask corrections)
    out[(b,i),o] = sum_c feat[(b,i),c] * KV[(b,i), c*32+o]
This cuts the W3-stage FLOPs by 16x. Bias/mask corrections are rank-1 in q
and folded into the KV PSUM accumulation as a K=2 matmul
(rows: nv*b3 and -(K-nv)*kv0, kv0 = MLP(0) @ W3), so results stay exact
for arbitrary b1/b2/b3 even though dt-masking zeroes invalid offsets.

Sharding: 8 cores = 2 batches x 4 sequence shards of 512 positions; each
shard's tB carries a 16-timestamp halo from the previous shard (host-staged).
"""

import sys

import numpy as np

sys.path.insert(0, "/opt/trn_rl_repo")

from concourse import bacc, bass, mybir, tile  # noqa: E402
from concourse.bass_utils import run_bass_kernel_spmd  # noqa: E402

BS, L, CIN, COUT, HID, K = 2, 2048, 32, 32, 128, 16
NCORES = 8
NSH = 4          # sequence shards per batch
SH = L // NSH    # positions per core (512)
NQT = SH // 128  # q-tiles per core (4)
F32 = mybir.dt.float32

_cache: dict = {}


def _enable_ldw_opt():
    """Let walrus dedup identical consecutive LDWEIGHTS (the default
    --enable-ldw-opt=false re-loads the stationary operand before every
    matmul; our phases reuse one stationary operand dozens of times)."""
    from concourse import bass_utils

    if getattr(bass_utils.run_command, "_ldw_patched", False):
        return
    orig = bass_utils.run_command

    def patched(cmd, *a, **kw):
        cmd = [
            c.replace("--enable-ldw-opt=false", "--enable-ldw-opt=true")
            if isinstance(c, str) else c
            for c in cmd
        ]
        return orig(cmd, *a, **kw)

    patched._ldw_patched = True
    bass_utils.run_command = patched


def _build_bass():
    """Build + compile the SPMD single-core Bass program (identical on all
    cores; per-core behavior comes entirely from the input tensors)."""
    _enable_ldw_opt()
    nc = bacc.Bacc("TRN2", target_bir_lowering=False, debug=False)

    R32 = mybir.dt.float32r  # fp32 bits, single-pass PE mode (1 cyc/row vs 4)
    dram = {}
    for name, shape, dt_ in [
        ("tA", (K, SH), F32),       # t_i broadcast over d rows
        ("tB", (K, SH), F32),       # t_{i-1-d}, halo-padded (clipped to t_0)
        ("mask16", (K, SH), F32),   # 1.0 where i-1-d >= 0
        ("featq", (128, NQT * CIN), F32),  # feat[q, t*32+c] (q-tile-major)
        ("W1r", (1, HID), R32),     # W1 row
        ("W2", (HID, HID), R32),
        ("W3", (HID, CIN * COUT), R32),
        ("b1c", (HID, 1), F32),
        ("b2c", (HID, 1), F32),
        ("eye", (HID, HID), R32),   # identity for the d-sum PSUM accumulation
        ("nvmat", (2, SH), R32),    # rows: nv, K-nv  (valid-offset counts)
        ("rhs2", (2, CIN * COUT), R32),  # rows: b3, -kv0
    ]:
        dram[name] = nc.dram_tensor(name, list(shape), dt_, kind="ExternalInput")
    out_dram = nc.dram_tensor("out", [SH, COUT], F32, kind="ExternalOutput")

    Relu = mybir.ActivationFunctionType.Relu
    Add = mybir.AluOpType.add
    Max = mybir.AluOpType.max
    Mult = mybir.AluOpType.mult



    NW = 1024  # wide tile: 2 d-offsets side by side (2 PSUM banks)

    with tile.TileContext(nc) as tc:
        with (
            tc.tile_pool(name="const", bufs=1) as const,
            tc.tile_pool(name="work", bufs=1) as work,
            tc.tile_pool(name="h1p", bufs=8) as h1p,
            tc.tile_pool(name="h2p", bufs=8) as h2p,
            tc.tile_pool(name="stage5", bufs=2) as s5p,
            # 2 pools x 2 bufs x [128,1024] = 8 PSUM banks total; the H
            # accumulator and KV tiles reuse these slots after the phases.
            tc.tile_pool(name="ps1", bufs=2, space=bass.MemorySpace.PSUM) as ps1,
            tc.tile_pool(name="ps2", bufs=2, space=bass.MemorySpace.PSUM) as ps2,
        ):
            # ---- load inputs, spread over engine DMA queues ----
            # ---- PE warm-up setup first: zero tile built on gpsimd before
            # its DMA queue work, so warm-up matmuls can start ~7us in ----
            wzf = work.tile([HID, SH], F32, tag="wzf")
            nc.gpsimd.memset(wzf[:], 0.0)
            wz = work.tile([HID, SH], R32, tag="wz")
            nc.gpsimd.tensor_copy(wz[:], wzf[:])

            # dt-critical inputs go first on their queues
            qeng = {
                "tA": nc.sync, "tB": nc.sync, "mask16": nc.sync,
                "W1r": nc.scalar, "b1c": nc.scalar, "W2": nc.scalar,
                "b2c": nc.scalar, "eye": nc.scalar,
                "W3": nc.gpsimd, "featq": nc.gpsimd,
                "nvmat": nc.gpsimd, "rhs2": nc.gpsimd,
            }
            sb = {}
            for name in qeng:
                t = const.tile(list(dram[name].shape), dram[name].dtype, tag=name)
                qeng[name].dma_start(t[:], dram[name].ap())
                sb[name] = t

            # warm-up matmuls: no data deps beyond wz, so they fill the
            # preamble's dead PE time and open the HAM clock gate
            for i in range(14):
                pw = ps1.tile([HID, NW], F32, tag="p1")
                nc.tensor.matmul(
                    pw[:, :SH], wz[:, :HID], wz[:], start=True, stop=True
                )

            # ---- dt = relu(tA - tB) * mask ----
            dtsub = work.tile([K, SH], F32, tag="dtsub")
            nc.vector.tensor_sub(dtsub[:], sb["tA"][:], sb["tB"][:])
            dt2 = work.tile([K, SH], R32, tag="dt2")
            nc.vector.scalar_tensor_tensor(
                dt2[:], dtsub[:], 0.0, sb["mask16"][:], op0=Max, op1=Mult
            )
            # gather all 16 d-rows into one partition-0 tile (matmul operands
            # must start at a 32-aligned partition): drow d = dtrow[:, d*SH:]
            dtrow = work.tile([1, K * SH], R32, tag="dtrow")
            nc.sync.dma_start(
                dtrow[:].rearrange("p (d q) -> p d q", d=K), dt2[:, :]
            )
            drows = [dtrow[:, d * SH : (d + 1) * SH] for d in range(K)]

            # expanded feature tiles for the f-contraction: f_exp[q, o*32+c] =
            # feat[q, c], materialized by the idle gpsimd so the per-tile
            # multiplies use contiguous access patterns
            fexps = []
            for t in range(NQT):
                fe = s5p.tile([128, CIN * COUT], F32, tag=f"fe{t}")
                nc.gpsimd.tensor_copy(
                    fe[:].rearrange("p (o c) -> p o c", c=CIN),
                    sb["featq"][:, t * CIN : (t + 1) * CIN]
                    .unsqueeze(1)
                    .broadcast_to([128, COUT, CIN]),
                )
                fexps.append(fe)

            # ---- per-offset MLP, phase-separated (constant stationary
            # operand per phase keeps the PE stream dense), processed in
            # d-pairs so relus run as wide [128,1024] ops ----
            # Phase A: h1_d = relu(W1 (x) dt_d + b1)
            h1s = []
            for p in range(K // 2):
                pA = ps1.tile([HID, NW], F32, tag="p1")
                for j in range(2):
                    nc.tensor.matmul(
                        pA[:, j * SH : (j + 1) * SH], sb["W1r"][:],
                        drows[2 * p + j], start=True, stop=True,
                    )
                h1 = h1p.tile([HID, NW], R32, tag="h1")
                if p in (0, 2, 4, 6):
                    nc.scalar.activation(h1[:], pA[:], Relu, bias=sb["b1c"][:])
                else:
                    nc.vector.tensor_scalar(
                        h1[:], pA[:], sb["b1c"][:], 0.0, op0=Add, op1=Max
                    )
                h1s.append(h1)
            # Phase B: h2_d = relu(W2.T @ h1_d + b2)
            h2s = []
            for p in range(K // 2):
                pB = ps2.tile([HID, NW], F32, tag="p2")
                for j in range(2):
                    nc.tensor.matmul(
                        pB[:, j * SH : (j + 1) * SH], sb["W2"][:],
                        h1s[p][:, j * SH : (j + 1) * SH], start=True, stop=True,
                    )
                h2 = h2p.tile([HID, NW], R32, tag="h2")
                if p in (1, 3, 5, 6, 7):
                    nc.scalar.activation(h2[:], pB[:], Relu, bias=sb["b2c"][:])
                else:
                    nc.vector.tensor_scalar(
                        h2[:], pB[:], sb["b2c"][:], 0.0, op0=Add, op1=Max
                    )
                h2s.append(h2)
            # Phase C: H = sum_d h2_d (identity matmuls accumulating in PSUM).
            # The accumulator reuses a ps1 slot (phase A is drained by now).
            pHw = ps1.tile([HID, NW], F32, tag="p1")
            pH = pHw[:, :SH]
            n = 0
            for p in range(K // 2):
                for j in range(2):
                    nc.tensor.matmul(
                        pH, sb["eye"][:], h2s[p][:, j * SH : (j + 1) * SH],
                        start=(n == 0), stop=(n == K - 1),
                    )
                    n += 1

            Hs = work.tile([HID, SH], R32, tag="Hs")
            nc.vector.tensor_copy(Hs[:], pH)

            # ---- KV = H^T @ W3 (+ rank-1 corrections), then f-contraction ----
            CO = CIN * COUT
            for t in range(NQT):
                qs = slice(t * 128, (t + 1) * 128)
                kv = (ps2 if t % 2 == 0 else ps1).tile(
                    [128, CO], F32, tag="p2" if t % 2 == 0 else "p1"
                )
                for half in range(2):
                    hs = slice(half * 512, half * 512 + 512)
                    nc.tensor.matmul(
                        kv[:, hs], Hs[:, qs], sb["W3"][:, hs],
                        start=True, stop=False,
                    )
                for half in range(2):
                    hs = slice(half * 512, half * 512 + 512)
                    nc.tensor.matmul(
                        kv[:, hs], sb["nvmat"][:, qs], sb["rhs2"][:, hs],
                        start=False, stop=True,
                    )
                # prod stored o-major: prod[q, o*32+c] = kv[q, c*32+o]*f[q,c]
                # so the c-reduction below reads contiguously. Tiles 0,1 run
                # the multiply on gpsimd (needs an SBUF copy of kv first,
                # done by the then-idle ACT engine) so the last tile's tail
                # stays on the faster DVE path.
                prod = s5p.tile([128, CO], F32, tag="prod")
                kvT = kv[:].rearrange("p (c o) -> p o c", o=COUT)
                prodv = prod[:].rearrange("p (o c) -> p o c", c=CIN)
                fev = fexps[t][:].rearrange("p (o c) -> p o c", c=CIN)
                if t < 2:
                    kvs = s5p.tile([128, CO], F32, tag="kvs")
                    nc.scalar.copy(kvs[:], kv[:])
                    nc.gpsimd.tensor_tensor(
                        prodv,
                        kvs[:].rearrange("p (c o) -> p o c", o=COUT),
                        fev, op=Mult,
                    )
                else:
                    nc.vector.tensor_tensor(prodv, kvT, fev, op=Mult)
                # out[q, o] = sum_c prod[q, o, c]
                ot = s5p.tile([128, COUT], F32, tag="ot")
                nc.vector.tensor_reduce(
                    ot[:],
                    prod[:].rearrange("p (o c) -> p o c", c=CIN),
                    axis=mybir.AxisListType.X,
                    op=Add,
                )
                nc.sync.dma_start(out_dram.ap()[qs, :], ot[:])

    nc.compile()
    return nc


def _stage_inputs(times, features, W1, b1, W2, b2, W3, b3):
    """Host-side staging: shard + precompute per-core input tensors."""
    times = np.ascontiguousarray(times, dtype=np.float32)
    features = np.ascontiguousarray(features, dtype=np.float32)
    W1 = np.asarray(W1, np.float32).reshape(1, HID)
    b1 = np.asarray(b1, np.float32).reshape(HID)
    W2 = np.asarray(W2, np.float32)
    b2 = np.asarray(b2, np.float32).reshape(HID)
    W3 = np.asarray(W3, np.float32)
    b3 = np.asarray(b3, np.float32).reshape(CIN * COUT)

    h2_0 = np.maximum(W2.T @ np.maximum(b1, 0.0) + b2, 0.0)
    kv0 = h2_0 @ W3
    rhs2 = np.ascontiguousarray(np.stack([b3, -kv0]).astype(np.float32))
    eye = np.eye(HID, dtype=np.float32)
    b1c = np.ascontiguousarray(b1[:, None])
    b2c = np.ascontiguousarray(b2[:, None])
    dd = np.arange(K)[:, None]

    in_maps = []
    for c in range(NCORES):
        b, s = divmod(c, NSH)
        gi = s * SH + np.arange(SH)
        src = gi[None, :] - 1 - dd
        in_maps.append({
            "tA": np.ascontiguousarray(np.broadcast_to(times[b, gi], (K, SH))),
            "tB": np.ascontiguousarray(times[b, np.clip(src, 0, L - 1)]),
            "mask16": (src >= 0).astype(np.float32),
            "featq": np.ascontiguousarray(
                features[b, gi].reshape(NQT, 128, CIN)
                .transpose(1, 0, 2).reshape(128, NQT * CIN)
            ),
            "W1r": W1,
            "W2": W2,
            "W3": W3,
            "b1c": b1c,
            "b2c": b2c,
            "eye": eye,
            "nvmat": np.ascontiguousarray(
                np.stack([np.minimum(gi, K), K - np.minimum(gi, K)])
            ).astype(np.float32),
            "rhs2": rhs2,
        })
    return in_maps


def kernel(times, features, W1, b1, W2, b2, W3, b3, kernel_size, **run_kwargs):
    assert int(kernel_size) == K
    assert times.shape == (BS, L) and features.shape == (BS, L, CIN)

    if "nc" not in _cache:
        _cache["nc"] = _build_bass()
    nc = _cache["nc"]

    in_maps = _stage_inputs(times, features, W1, b1, W2, b2, W3, b3)
    res = run_bass_kernel_spmd(
        nc, in_maps, core_ids=list(range(NCORES)), **run_kwargs
    )

    out = np.empty((BS, L, COUT), np.float32)
    for c in range(NCORES):
        b, s = divmod(c, NSH)
        out[b, s * SH : (s + 1) * SH, :] = res.results[c]["out"]
    if run_kwargs:
        _cache["last_results"] = res
    return out
